# revision 1
# baseline (speedup 1.0000x reference)
"""Decode-step multi-head attention with KV cache (DeepSpeed-inference style).

Full shapes (hardcoded per problem spec):
  query/key/value: [16, 1, 2048] f32
  key_cache/value_cache: [16, 16, 4096, 128] f32
  cache_len: scalar int (2048)
Output: [16, 1, 2048] f32

Strategy: data-parallel over batch across 8 NeuronCores (2 batches/core =
32 (batch, head) pairs per core). Per pair, the core streams the K and V
cache slices ([cache_len, 128] each) from HBM, computes scores with
multiply+reduce on VectorE (K stays in its natural [k, d] layout), exp via
ScalarE (with fused row-sum for the softmax denominator), and aggregates
V with TensorE matmuls (contraction over the k partition axis). The new
token's score/value is folded in as an extra column / extra matmul.

The kernel is bound by the 16 DMA engines' HBM read side (~24 GB/s per
engine on 8KB packets; K+V cache = 64MiB/core), so every engine packet is
a mandatory cache read and every compute engine needs enough slack never
to stall the stream, even on cores with slow engine clocks (observed ~25%
run-to-run engine-speed variance):

- K rides the sync (SP) HWDGE queue in f32; V rides the gpsimd SWDGE
  queue cast f32->f16 in flight (free on the DMA-engine read side, and PE
  f16 weights are ~10x faster to load than f32).
- ScalarE casts each K tile to f16 so the score multiply+reduce runs at
  16-bit VectorE throughput (~2.2us/pair vs ~4.6 in f32) - both engines
  end up with >2x slack against the DMA stream.
- Small setup loads ride the scalar (ACT) HWDGE queue; queries are
  replicated across partitions with a PE outer product (ones x q_row),
  not a 2MB broadcast DMA.
- The last pair streams in quarters whose tiles reuse the stream pool
  tags, so pool-slot dependencies pin those DMAs to the stream's end and
  each quarter's compute overlaps the next quarter's DMA, leaving only a
  quarter-pair serial chain after the final packet.
"""

import functools
import os
from contextlib import ExitStack

import numpy as np

import concourse.bacc as bacc
import concourse.bass as bass
import concourse.mybir as mybir
import concourse.tile as tile
from concourse import bass_utils

N_CORES = 8
P = 128  # partitions

# test.py hooks: set TRACE=True before calling kernel() to collect a profile.
TRACE = False
TRACE_KWARGS = {}
LAST_RESULTS = None


def _build_program(bl: int, n_heads: int, max_seq: int, hd: int, cache_len: int):
    """Build + compile the per-core program. bl = local batch count."""
    npairs = bl * n_heads
    assert hd == P
    nch = cache_len // P          # full 128-row chunks of the cache
    rem = cache_len - nch * P     # remainder rows
    assert rem == 0, "cache_len % 128 != 0 not needed for this problem"
    ncht = nch
    sm_scale = 1.0 / float(np.sqrt(hd))
    # stream the last pair in quarters so its compute overlaps its own DMA
    tail_halves = nch % 4 == 0 and nch >= 8 and npairs >= 4

    nc = bacc.Bacc("TRN2", target_bir_lowering=False, debug=False)
    f32 = mybir.dt.float32
    f16 = mybir.dt.float16

    kc = nc.dram_tensor("kc", [bl, n_heads, max_seq, hd], f32, kind="ExternalInput").ap()
    vc = nc.dram_tensor("vc", [bl, n_heads, max_seq, hd], f32, kind="ExternalInput").ap()
    q = nc.dram_tensor("q", [npairs, hd], f32, kind="ExternalInput").ap()
    kn = nc.dram_tensor("kn", [npairs, hd], f32, kind="ExternalInput").ap()
    vn = nc.dram_tensor("vn", [npairs, hd], f32, kind="ExternalInput").ap()
    ident = nc.dram_tensor("ident", [P, P], f32, kind="ExternalInput").ap()
    out = nc.dram_tensor("out", [npairs, hd], f32, kind="ExternalOutput").ap()

    with tile.TileContext(nc) as tc, ExitStack() as ctx:
        singles = ctx.enter_context(tc.tile_pool(name="singles", bufs=1))
        kbufs = int(os.environ.get("KBUFS", "8"))
        kpool = ctx.enter_context(tc.tile_pool(name="kpool", bufs=kbufs))
        vpool = ctx.enter_context(tc.tile_pool(name="vpool", bufs=kbufs))
        k16pool = ctx.enter_context(tc.tile_pool(name="k16pool", bufs=2))
        ppool = ctx.enter_context(tc.tile_pool(name="ppool", bufs=2))
        stats = ctx.enter_context(tc.tile_pool(name="stats", bufs=4))
        psum_o = ctx.enter_context(tc.tile_pool(name="psum_o", bufs=2, space="PSUM"))
        psum_1 = ctx.enter_context(tc.tile_pool(name="psum_1", bufs=2, space="PSUM"))

        def emit_loads(b, h):
            kt = kpool.tile([P, ncht, hd], f32, tag="kt")
            # V is cast to fp16 during the DMA (SWDGE): free on the DMA
            # engine read side, and PE loads f16 weights ~10x faster.
            vt = vpool.tile([P, ncht, hd], f16, tag="vt")
            kslc = kc[b, h, 0 : nch * P, :].rearrange("(p c) d -> p c d", c=nch)
            vslc = vc[b, h, 0 : nch * P, :].rearrange("(p c) d -> p c d", c=nch)
            nc.sync.dma_start(out=kt, in_=kslc)
            nc.gpsimd.dma_start(out=vt, in_=vslc)
            return kt, vt

        # issue the first pairs' streaming loads before any setup traffic so
        # the sync queue's first instruction is a K DMA and the gpsimd
        # queue's first work is V SWDGE generation
        n_stream = npairs - 1 if tail_halves else npairs
        PRELOAD = min(3, n_stream)
        preloaded = [emit_loads(*divmod(p, n_heads)) for p in range(PRELOAD)]

        ones_col = singles.tile([P, 1], f32)
        nc.vector.memset(ones_col, 1.0)

        # small setup loads ride the scalar (ACT) HWDGE queue so they never
        # delay the K/V streams
        q_row = singles.tile([1, npairs * hd], f32)
        q_row_src = bass.AP(
            tensor=q.tensor, offset=q.offset, ap=[[0, 1], [1, npairs * hd]]
        )
        nc.scalar.dma_start(out=q_row, in_=q_row_src)
        kn_all = singles.tile([npairs, hd], f32)
        nc.scalar.dma_start(out=kn_all, in_=kn)
        vn_all = singles.tile([npairs, hd], f32)
        nc.scalar.dma_start(out=vn_all, in_=vn)
        q_all = singles.tile([npairs, hd], f32)
        nc.scalar.dma_start(out=q_all, in_=q)
        ident_sb = singles.tile([P, P], f32)
        nc.scalar.dma_start(out=ident_sb, in_=ident)

        # all queries broadcast to every partition, once, as a PE outer
        # product ones[1,128] x q_row[1,*] (not DMA: a 2MB broadcast DMA
        # costs ~6.5us of DMA engine time; not gpsimd partition_broadcast:
        # that would serialize the Pool engine against V SWDGE generation).
        # f16 replicas feed the 16-bit score path.
        ones_row = singles.tile([1, P], f32)
        nc.vector.memset(ones_row, 1.0)
        q_all_b = singles.tile([P, npairs, hd], f16)
        GPAIRS = 4  # pairs per chunk; 4*hd f32 = one 2KB PSUM bank
        ngrp = npairs // GPAIRS
        for g in range(ngrp):
            qb_ps = psum_1.tile([P, GPAIRS, hd], f32, tag="qb")
            qb_2d = bass.AP(
                tensor=qb_ps.tensor,
                offset=qb_ps.offset,
                ap=[qb_ps.ap[0], [1, GPAIRS * hd]],
            )
            nc.tensor.matmul(
                qb_2d,
                lhsT=ones_row,
                rhs=q_row[0:1, g * GPAIRS * hd : (g + 1) * GPAIRS * hd],
                start=True,
                stop=True,
            )
            nc.scalar.copy(q_all_b[:, g * GPAIRS : (g + 1) * GPAIRS, :], qb_ps)

        # Softmax denominators, one column per pair (partition 0).
        lrow = psum_1.tile([1, npairs], f32, tag="l")
        # Unnormalized cache-part outputs, head-dim on partitions, one
        # column per pair.
        out_sb = singles.tile([P, npairs], f32)

        # ---- new-token contribution, batched over all pairs ----
        prod_new = singles.tile([npairs, hd], f32)
        nc.vector.tensor_mul(prod_new, kn_all, q_all)
        s_new = singles.tile([npairs, 1], f32)
        nc.vector.reduce_sum(s_new, prod_new, axis=mybir.AxisListType.X)
        p_new = singles.tile([npairs, 1], f32)
        nc.scalar.activation(
            out=p_new, in_=s_new, func=mybir.ActivationFunctionType.Exp, scale=sm_scale
        )
        # rows 0..npairs-1: p_new[p] * v_new[p]; rest zero
        vns = singles.tile([P, hd], f32)
        nc.vector.memset(vns, 0.0)
        nc.vector.tensor_scalar_mul(vns[:npairs, :], vn_all, p_new)
        vnsT_ps = psum_1.tile([P, P], f32, tag="wide")
        nc.tensor.transpose(vnsT_ps, vns, ident_sb)
        vnsT = singles.tile([P, npairs], f32)
        nc.scalar.copy(vnsT, vnsT_ps[:, :npairs])

        def bcast(ap2d, nb):
            return bass.AP(
                tensor=ap2d.tensor,
                offset=ap2d.offset,
                ap=[ap2d.ap[0], [0, nb], ap2d.ap[1]],
            )

        # lrow starts as p_new^T (one transpose matmul); each pair then
        # accumulates its denominator into its column (start=False), so the
        # epilogue needs no separate p_new add
        nc.tensor.matmul(
            lrow, lhsT=p_new, rhs=ident_sb[:npairs, :npairs], start=True, stop=True
        )

        def pair_tail_ops(pr, lblk, nblk, acc):
            if nblk == 1:
                l_part = lblk
            else:
                l_part = stats.tile([P, 1], f32, tag="l")
                nc.vector.reduce_sum(l_part, lblk, axis=mybir.AxisListType.X)
            nc.tensor.matmul(
                lrow[0:1, pr : pr + 1], lhsT=ones_col, rhs=l_part, start=False, stop=True
            )
            # fold the new-token column in here instead of one big epilogue add
            nc.vector.tensor_add(out_sb[:, pr : pr + 1], acc, vnsT[:, pr : pr + 1])

        for p in range(n_stream):
            b, h = divmod(p, n_heads)
            kt, vt = preloaded[p] if p < len(preloaded) else emit_loads(b, h)

            # ScalarE casts K to f16 so DVE runs the multiply at 16-bit (2x)
            # throughput
            kt16 = k16pool.tile([P, ncht, hd], f16, tag="kt16")
            nc.scalar.copy(kt16, kt)

            prod = ppool.tile([P, ncht, hd], f16, tag="prod")
            s_tile = stats.tile([P, ncht], f32, tag="s")
            nc.vector.tensor_mul(prod, kt16, bcast(q_all_b[:, p, :], ncht))
            # fold products pairwise with f16 adds (2 elem/cycle) so the
            # 1-elem/cycle reduce only sees a quarter of the elements
            fold1 = ppool.tile([P, ncht, hd // 2], f16, tag="f1")
            nc.vector.tensor_add(fold1, prod[:, :, : hd // 2], prod[:, :, hd // 2 :])
            fold2 = ppool.tile([P, ncht, hd // 4], f16, tag="f2")
            nc.vector.tensor_add(fold2, fold1[:, :, : hd // 4], fold1[:, :, hd // 4 :])
            nc.vector.reduce_sum(s_tile, fold2, axis=mybir.AxisListType.X)

            p_tile = stats.tile([P, ncht], f16, tag="p")
            l_part = stats.tile([P, 1], f32, tag="l")
            nc.scalar.activation(
                out=p_tile,
                in_=s_tile,
                func=mybir.ActivationFunctionType.Exp,
                scale=sm_scale,
                accum_out=l_part,
            )

            acc = psum_o.tile([P, 1], f32, tag="acc")
            for c in range(ncht):
                nc.tensor.matmul(
                    acc,
                    lhsT=vt[:, c, :],
                    rhs=p_tile[:, c : c + 1],
                    start=(c == 0),
                    stop=(c == ncht - 1),
                )
            pair_tail_ops(p, l_part, 1, acc)

        if tail_halves:
            # Last pair in quarters. Its K/V tiles reuse the stream pool
            # tags, so pool-slot dependencies pin these DMAs behind the
            # earlier pairs' loads at the stream's end; each quarter's
            # compute (same cast+fold f16 path as the main loop, per
            # quarter) overlaps the next quarter's DMA.
            NBLK = 4
            nq = nch // NBLK
            p = npairs - 1
            b, h = divmod(p, n_heads)
            kt = kpool.tile([P, ncht, hd], f32, tag="kt")
            vt = vpool.tile([P, ncht, hd], f16, tag="vt")
            # DMA in halves (4KB partition lines: near-8KB packet efficiency)
            # but compute in quarters - subtile deps gate each compute block
            # on just the half-DMA that covers it
            rows = (nch // 2) * P
            for hi in range(2):
                r0 = hi * rows
                kslc = kc[b, h, r0 : r0 + rows, :].rearrange(
                    "(p c) d -> p c d", c=nch // 2
                )
                vslc = vc[b, h, r0 : r0 + rows, :].rearrange(
                    "(p c) d -> p c d", c=nch // 2
                )
                cs = slice(hi * (nch // 2), (hi + 1) * (nch // 2))
                nc.sync.dma_start(out=kt[:, cs, :], in_=kslc)
                nc.gpsimd.dma_start(out=vt[:, cs, :], in_=vslc)

            acc = psum_o.tile([P, 1], f32, tag="acc")
            for qi in range(NBLK):
                cs = slice(qi * nq, (qi + 1) * nq)
                s_h = stats.tile([P, nq], f32, tag="sh")
                # cast+fold f16 path for every block: DVE (not hop latency)
                # is the serial resource here, and the casts hide on ScalarE
                # behind the previous block's DVE work
                kt16q = k16pool.tile([P, nq, hd], f16, tag="kt16q")
                nc.scalar.copy(kt16q, kt[:, cs, :])
                prodh = ppool.tile([P, nq, hd], f16, tag="prodh")
                nc.vector.tensor_mul(prodh, kt16q, bcast(q_all_b[:, p, :], nq))
                foldq1 = ppool.tile([P, nq, hd // 2], f16, tag="fq1")
                nc.vector.tensor_add(
                    foldq1, prodh[:, :, : hd // 2], prodh[:, :, hd // 2 :]
                )
                foldq2 = ppool.tile([P, nq, hd // 4], f16, tag="fq2")
                nc.vector.tensor_add(
                    foldq2, foldq1[:, :, : hd // 4], foldq1[:, :, hd // 4 :]
                )
                nc.vector.reduce_sum(s_h, foldq2, axis=mybir.AxisListType.X)
                p_h = stats.tile([P, nq], f16, tag="ph")
                l_blk = stats.tile([P, 1], f32, tag="l")
                nc.scalar.activation(
                    out=p_h,
                    in_=s_h,
                    func=mybir.ActivationFunctionType.Exp,
                    scale=sm_scale,
                    accum_out=l_blk,
                )
                # accumulate this block's denominator straight into the
                # pair's lrow column (no cross-block l reduce at the end)
                nc.tensor.matmul(
                    lrow[0:1, p : p + 1],
                    lhsT=ones_col,
                    rhs=l_blk,
                    start=False,
                    stop=(qi == NBLK - 1),
                )
                for c in range(nq):
                    nc.tensor.matmul(
                        acc,
                        lhsT=vt[:, qi * nq + c, :],
                        rhs=p_h[:, c : c + 1],
                        start=(qi == 0 and c == 0),
                        stop=(qi == NBLK - 1 and c == nq - 1),
                    )
            nc.vector.tensor_add(out_sb[:, p : p + 1], acc, vnsT[:, p : p + 1])

        # ---- epilogue: normalize, emit ----
        # transpose the denominator row [1, npairs] -> [npairs, 1] with one
        # tiny PE matmul (lhsT = the row itself, rhs = a single one)
        lrow_sb = singles.tile([1, npairs], f32)
        nc.scalar.copy(lrow_sb, lrow)
        lT_ps = psum_1.tile([npairs, 1], f32, tag="l")
        nc.tensor.matmul(
            lT_ps, lhsT=lrow_sb, rhs=ones_col[0:1, 0:1], start=True, stop=True
        )
        recip_l = singles.tile([npairs, 1], f32)
        nc.vector.reciprocal(recip_l, lT_ps)

        oT = psum_1.tile([npairs, hd], f32, tag="wide")
        nc.tensor.transpose(oT, out_sb, ident_sb)

        final_sb = singles.tile([npairs, hd], f32)
        nc.scalar.mul(final_sb, oT, mul=recip_l)
        nc.scalar.dma_start(out=out, in_=final_sb)

    nc.compile()
    return nc


@functools.lru_cache(maxsize=4)
def _program(bl, n_heads, max_seq, hd, cache_len):
    return _build_program(bl, n_heads, max_seq, hd, cache_len)


def kernel(query, key, value, key_cache, value_cache, cache_len):
    global LAST_RESULTS
    query = np.asarray(query, dtype=np.float32)
    key = np.asarray(key, dtype=np.float32)
    value = np.asarray(value, dtype=np.float32)
    key_cache = np.asarray(key_cache, dtype=np.float32)
    value_cache = np.asarray(value_cache, dtype=np.float32)
    cache_len = int(cache_len)

    b_sz, q_len, d_model = query.shape
    _, n_heads, max_seq, hd = key_cache.shape
    assert q_len == 1 and d_model == n_heads * hd
    assert b_sz % N_CORES == 0
    bl = b_sz // N_CORES

    prog = _program(bl, n_heads, max_seq, hd, cache_len)

    ident = np.eye(P, dtype=np.float32)
    in_maps = []
    for i in range(N_CORES):
        sl = slice(i * bl, (i + 1) * bl)
        in_maps.append(
            {
                "kc": np.ascontiguousarray(key_cache[sl]),
                "vc": np.ascontiguousarray(value_cache[sl]),
                "q": np.ascontiguousarray(query[sl]).reshape(bl * n_heads, hd),
                "kn": np.ascontiguousarray(key[sl]).reshape(bl * n_heads, hd),
                "vn": np.ascontiguousarray(value[sl]).reshape(bl * n_heads, hd),
                "ident": ident,
            }
        )

    try:
        res = bass_utils.run_bass_kernel_spmd(
            prog, in_maps, core_ids=list(range(N_CORES)), trace=TRACE, **TRACE_KWARGS
        )
    except Exception:
        # A previously crashed NeuronCore can leave the first execution
        # attempt failing with a transient runtime error; retry once.
        res = bass_utils.run_bass_kernel_spmd(
            prog, in_maps, core_ids=list(range(N_CORES)), trace=TRACE, **TRACE_KWARGS
        )
    LAST_RESULTS = res
    outs = [res.results[i]["out"].reshape(bl, q_len, d_model) for i in range(N_CORES)]
    return np.concatenate(outs, axis=0)

